# revision 1
# baseline (speedup 1.0000x reference)
"""Trainium2 Bass kernel for nn_AppPreUserPGtrDocAttn (sparse_attention).

Strategy: pure data-parallel over the window dim N across 8 NeuronCores.
Each core computes 512 output windows (last core: 509 real + 3 discarded).
All weights are replicated; inputs are sharded/padded/transposed on host.
Compute dtype: bf16 matmuls with fp32 PSUM accumulation.

Per-core pipeline (feature-major / transposed activations throughout):
  A: xT[0:256, :]  = emb_app_w.T @ app_shard.T      (K=10000 streamed)
     xT[256:320,:] = emb_tim_w.T @ onehot(tim)      (one-hot gather matmul)
  B: s = attn_W.T @ xT                              ([1, 515] row vector)
  C: H[f] = tanh(s[f:f+512] + b[f]); w[f] = H[f]/sum_f|H[f]|
  D: yT = attn_fc_w @ xT;  outT = sum_f bcast(w[f]) * yT[:, f:f+512]
  E: out2T[0:256] = (outT + fc_b) * uid_emb; out2T[256:320] = ptim one-hot
     out2T[320] = 1  (bias row; dec_b is appended to dec_w.T on host)
  F: score = sigmoid(dec_w_aug.T.T @ out2T) streamed over 10240 cols
"""

import numpy as np

try:
    import concourse.bass as bass
except ImportError:  # pragma: no cover
    import sys

    sys.path.insert(0, "/opt/trn_rl_repo")
    import concourse.bass as bass

import ml_dtypes

import concourse.mybir as mybir
from concourse import bacc, bass_utils
from concourse import tile
from concourse.tile import TileContext

BF = ml_dtypes.bfloat16
F32 = mybir.dt.float32
BF16 = mybir.dt.bfloat16
FP8 = mybir.dt.float8e4
F8 = ml_dtypes.float8_e4m3
AF = mybir.ActivationFunctionType
ALU = mybir.AluOpType

S = 4096            # sequence length
NWIN = S - 3        # 4093 windows
NCORES = 8
R = 512             # windows per core (last core: 509 real)
RH = R + 3          # x rows needed per core (halo)
RP = 520            # padded col count for xT/appT (512 + 8)
KAPP = 10000        # app vocab / contraction dim
KAPPP = 10240       # padded to 80 k-tiles of 128
NKT = KAPPP // 128  # 80
KB = 8              # k-tiles per DMA batch
NKB = NKT // KB     # 10
E = 256             # app emb dim
TE = 64             # tim emb dim
D = 320             # INPUT_SIZE
DP = 384            # padded feature dim (3 k-tiles of 128)
NOUT = 10000        # decoder outputs
NOUTP = 10240       # padded to 20 chunks of 512
GW = 2048           # out cols per group (4 chunks of 512)
NG = NOUTP // GW    # 5

_CACHE: dict = {}


def _build():
    nc = bacc.Bacc()

    appT_d = nc.declare_dram_parameter("appT", [KAPPP, RP], FP8, isOutput=False)
    wapp_d = nc.declare_dram_parameter("wapp", [KAPPP, E], FP8, isOutput=False)
    decw_d = nc.declare_dram_parameter("decw", [DP, NOUTP], FP8, isOutput=False)
    fcw_d = nc.declare_dram_parameter("fcw", [DP, E], BF16, isOutput=False)
    embt_d = nc.declare_dram_parameter("embt", [48, TE], BF16, isOutput=False)
    attnwr_d = nc.declare_dram_parameter("attnwr", [DP, 128], BF16, isOutput=False)
    timv_d = nc.declare_dram_parameter("timv", [RP], BF16, isOutput=False)
    ptimv_d = nc.declare_dram_parameter("ptimv", [R], BF16, isOutput=False)
    # constf cols: 0 iota, 1:3 uid_emb, 3:5 fc_b, 5:9 attn_b (all [128,1] views)
    constf_d = nc.declare_dram_parameter("constf", [128, 9], F32, isOutput=False)
    out_d = nc.declare_dram_parameter("out", [R, NOUTP], BF16, isOutput=True)

    with TileContext(nc) as tc:
        with (
            tc.tile_pool(name="const", bufs=1) as const,
            tc.tile_pool(name="sb", bufs=1) as sb,
            tc.tile_pool(name="apool", bufs=3) as apool,
            tc.tile_pool(name="wpool", bufs=3) as wpool,
            tc.tile_pool(name="dpool", bufs=5) as dpool,
            tc.tile_pool(name="opool", bufs=3) as opool,
            tc.tile_pool(name="tmp", bufs=2) as tmp,
        ):
            # ---- constants / small inputs ----
            ones_sb = const.tile([1, 128], BF16)
            nc.vector.memset(ones_sb[:], 1.0)
            constf_sb = const.tile([128, 9], F32)
            nc.sync.dma_start(constf_sb[:], constf_d[:, :])
            timv_sb = const.tile([1, RP], BF16)
            nc.sync.dma_start(timv_sb[:], timv_d.rearrange("(o c) -> o c", o=1))
            ptimv_sb = const.tile([1, R], BF16)
            nc.sync.dma_start(ptimv_sb[:], ptimv_d.rearrange("(o c) -> o c", o=1))
            embt_sb = const.tile([48, TE], BF16)
            nc.sync.dma_start(embt_sb[:], embt_d[:, :])
            attnwr_sb = const.tile([128, 3, 128], BF16)
            nc.sync.dma_start(attnwr_sb[:], attnwr_d.rearrange("(t p) m -> p t m", p=128))
            fcw_sb = const.tile([128, 3, E], BF16)
            nc.sync.dma_start(fcw_sb[:], fcw_d.rearrange("(t p) e -> p t e", p=128))
            iota_sb = constf_sb[:, 0:1]

            # pre-warm ACT tables off the critical path
            warm = const.tile([1, 1], F32)
            nc.vector.memset(warm[:], 0.5)
            nc.scalar.activation(warm[:], warm[:], AF.Tanh)
            nc.scalar.activation(warm[:], warm[:], AF.Sigmoid)

            # persistent activations
            xTa = sb.tile([128, 2, RP], BF16)      # x.T features 0:256
            xTt = sb.tile([TE, RP], BF16)          # x.T features 256:320
            H_b = sb.tile([128, 4, R], F32)        # tanh windows, bcast over P
            rec = sb.tile([128, R], F32)           # 1/L1, bcast over P
            yT = sb.tile([128, 2, RP], F32)        # fc_w @ x.T
            o2a = sb.tile([128, 2, R], FP8)        # out2.T rows 0:256, x64
            o2t = sb.tile([128, R], FP8)           # out2.T rows 256:384, x64

            # ---- tim / ptim one-hot embedding gathers (runs during the
            #      stage-A DMA preamble: only needs the tiny const loads).
            #      psT and psA are open together so they get disjoint PSUM
            #      banks (4 + 4) and the tim block overlaps stage A. ----
            with (
                tc.tile_pool(name="psT", bufs=1, space="PSUM") as psT,
                tc.tile_pool(name="psA", bufs=1, space="PSUM") as psA,
            ):
                pb = psT.tile([48, 512], F32)
                pt = psT.tile([TE, 512], F32)
                ppb = psT.tile([48, 512], F32)
                ppt = psT.tile([TE, 512], F32)
                oh = tmp.tile([48, RP], BF16, name="oh")
                ohp = tmp.tile([48, R], BF16, name="ohp")

                nc.tensor.matmul(pb[:], ones_sb[0:1, 0:48], timv_sb[0:1, 0:512],
                                 start=True, stop=True)
                nc.vector.tensor_scalar(oh[:, 0:512], pb[:], iota_sb[0:48, :],
                                        None, op0=ALU.is_equal)
                nc.tensor.matmul(pb[0:48, 0:8], ones_sb[0:1, 0:48],
                                 timv_sb[0:1, 512:RP], start=True, stop=True)
                nc.vector.tensor_scalar(oh[:, 512:RP], pb[0:48, 0:8],
                                        iota_sb[0:48, :], None, op0=ALU.is_equal)
                nc.tensor.matmul(pt[:], embt_sb[:], oh[:, 0:512],
                                 start=True, stop=True)
                nc.vector.tensor_copy(xTt[:, 0:512], pt[:])
                nc.tensor.matmul(pt[0:TE, 0:8], embt_sb[:], oh[:, 512:RP],
                                 start=True, stop=True)
                nc.vector.tensor_copy(xTt[:, 512:RP], pt[0:TE, 0:8])

                nc.tensor.matmul(ppb[:], ones_sb[0:1, 0:48], ptimv_sb[:],
                                 start=True, stop=True)
                nc.vector.tensor_scalar(ohp[:], ppb[:], iota_sb[0:48, :],
                                        None, op0=ALU.is_equal)
                nc.tensor.matmul(ppt[:], embt_sb[:], ohp[:],
                                 start=True, stop=True)
                nc.scalar.mul(o2t[0:TE, :], ppt[:], 64.0)
                nc.vector.memset(o2t[TE:128, :], 0.0)
                nc.vector.memset(o2t[TE:TE + 1, :], 64.0)  # bias row (dec_b)

                # ---- stage A: xT[0:256] = wapp.T @ appT, streamed over K ----
                BATCHES = [2, 2, 4] + [KB] * 9
                assert sum(BATCHES) == NKT
                pxa0 = psA.tile([128, 512], F32)
                pxa1 = psA.tile([128, 512], F32)
                px80 = psA.tile([128, 8], F32)
                px81 = psA.tile([128, 8], F32)
                pxa = [pxa0, pxa1]
                px8 = [px80, px81]
                appT_r = appT_d.rearrange("(kt p) c -> p kt c", p=128)
                wapp_r = wapp_d.rearrange("(kt p) e -> p kt e", p=128)
                kt0 = 0
                for nb in BATCHES:
                    at = apool.tile([128, KB, RP], FP8, name="at")
                    wt = wpool.tile([128, KB, E], FP8, name="wt")
                    at_dma = nc.sync.dma_start(at[:, 0:nb, :],
                                               appT_r[:, kt0:kt0 + nb, :])
                    nc.sync.dma_start(wt[:, 0:nb, :], wapp_r[:, kt0:kt0 + nb, :])
                    for k in range(nb):
                        kt = kt0 + k
                        start = kt == 0
                        stop = kt == NKT - 1
                        for mt in range(2):
                            lhsT = wt[:, k, mt * 128:(mt + 1) * 128]
                            nc.tensor.matmul(pxa[mt][:], lhsT, at[:, k, 0:512],
                                             start=start, stop=stop)
                            nc.tensor.matmul(px8[mt][:], lhsT, at[:, k, 512:RP],
                                             start=start, stop=stop)
                    kt0 += nb
                for mt in range(2):
                    nc.vector.tensor_scalar_mul(xTa[:, mt, 0:512], pxa[mt][:],
                                                1.0 / 16.0)
                    nc.vector.tensor_scalar_mul(xTa[:, mt, 512:RP], px8[mt][:],
                                                1.0 / 16.0)

            # ---- stage B/C: yT = fc_w @ xT ; Bs[f][p,n] = s[n+f] via
            #      replicated attn_W lhsT (no 1-partition ops anywhere) ----
            with tc.tile_pool(name="psB", bufs=1, space="PSUM") as psB:
                py0 = psB.tile([128, 512], F32)
                py08 = psB.tile([128, 8], F32)
                py1 = psB.tile([128, 512], F32)
                py18 = psB.tile([128, 8], F32)
                py = [py0, py1]
                py8 = [py08, py18]
                pw = [psB.tile([128, R], F32, name=f"pw{f}") for f in range(4)]
                xts = [xTa[:, 0, :], xTa[:, 1, :], xTt[:, :]]
                klens = [128, 128, TE]
                for kt in range(3):
                    xt, kl = xts[kt], klens[kt]
                    st, sp = kt == 0, kt == 2
                    for f in range(4):
                        nc.tensor.matmul(pw[f][:], attnwr_sb[0:kl, kt, :],
                                         xt[0:kl, f:f + R], start=st, stop=sp)
                    for mt in range(2):
                        lhsT = fcw_sb[0:kl, kt, mt * 128:(mt + 1) * 128]
                        nc.tensor.matmul(py[mt][:], lhsT, xt[0:kl, 0:512],
                                         start=st, stop=sp)
                        nc.tensor.matmul(py8[mt][:], lhsT, xt[0:kl, 512:RP],
                                         start=st, stop=sp)
                for f in range(4):
                    nc.scalar.activation(H_b[:, f, :], pw[f][:], AF.Tanh,
                                         bias=constf_sb[:, 5 + f:6 + f])
                nc.vector.tensor_copy(yT[:, 0, 0:512], py[0][:])
                nc.vector.tensor_copy(yT[:, 0, 512:RP], py8[0][:])
                nc.scalar.copy(yT[:, 1, 0:512], py[1][:])
                nc.scalar.copy(yT[:, 1, 512:RP], py8[1][:])
                l1 = tmp.tile([128, R], F32, name="l1")
                nc.vector.tensor_reduce(
                    l1[:], H_b.rearrange("p f n -> p n f"),
                    mybir.AxisListType.X, ALU.add, apply_absolute_value=True)
                nc.vector.reciprocal_approx_fast(rec[:], l1[:])

                # ---- stage D/E: outT = (sum_f H[f] * yT[:, f:f+R]) * rec ----
                for mt, eng in ((0, nc.vector), (1, nc.gpsimd)):
                    acc = tmp.tile([128, R], F32, name="acc")
                    prod = tmp.tile([128, R], F32, name="prod")
                    eng.tensor_mul(acc[:], H_b[:, 0, :], yT[:, mt, 0:R])
                    for f in range(1, 4):
                        eng.tensor_mul(prod[:], H_b[:, f, :],
                                       yT[:, mt, f:f + R])
                        eng.tensor_add(acc[:], acc[:], prod[:])
                    eng.tensor_mul(acc[:], acc[:], rec[:])
                    eng.tensor_scalar(o2a[:, mt, :], acc[:],
                                      constf_sb[:, 3 + mt:4 + mt],
                                      constf_sb[:, 1 + mt:2 + mt],
                                      op0=ALU.add, op1=ALU.mult)

            # ---- stage F: score = sigmoid(out2T.T @ decw) ----
            with tc.tile_pool(name="psF", bufs=4, space="PSUM") as psF:
                o2 = [o2a[:, 0, :], o2a[:, 1, :], o2t[:, :]]
                decw_r = decw_d.rearrange("(t p) (g c) -> g p t c", p=128, c=GW)
                for g in range(NG):
                    dw = dpool.tile([128, 3, GW], FP8, name="dw")
                    dw_dma = nc.sync.dma_start(dw[:], decw_r[g])
                    tile.add_dep_helper(
                        dw_dma.ins, at_dma.ins, sync=True,
                        reason="defer dec stream until stage-A input stream done")
                    for mt in range(4):
                        ob = opool.tile([128, GW], BF16, name="ob")
                        for sub in range(4):
                            pf = psF.tile([128, 512], F32, name="pf")
                            for kt in range(3):
                                nc.tensor.matmul(
                                    pf[:],
                                    o2[kt][:, mt * 128:(mt + 1) * 128],
                                    dw[:, kt, sub * 512:(sub + 1) * 512],
                                    start=(kt == 0), stop=(kt == 2))
                            nc.scalar.activation(ob[:, sub * 512:(sub + 1) * 512],
                                                 pf[:], AF.Sigmoid,
                                                 scale=1.0 / 1024.0)
                        nc.sync.dma_start(
                            out_d[mt * 128:(mt + 1) * 128, g * GW:(g + 1) * GW],
                            ob[:])

    nc.finalize()
    return nc


def _host_prep(tim, app, uid, ptim, emb_tim_w, emb_uid_w, emb_app_w,
               attn_W, attn_b, attn_fc_w, attn_fc_b, dec_w, dec_b):
    """Shard + pad + transpose + cast all inputs; returns in_maps for 8 cores."""
    app = np.asarray(app, dtype=np.float32)
    tim = np.asarray(tim).reshape(-1)
    ptim = np.asarray(ptim).reshape(-1)
    uid = int(np.asarray(uid).reshape(-1)[0])

    app_f8 = app.astype(F8)

    wapp = np.zeros((KAPPP, E), dtype=F8)
    wapp[:KAPP] = (np.asarray(emb_app_w, dtype=np.float32) * 16.0).astype(F8)

    decw = np.zeros((DP, NOUTP), dtype=F8)
    dwT = np.ascontiguousarray(np.asarray(dec_w, dtype=np.float32).T)  # [320, 10000]
    decw[:D, :NOUT] = (dwT * 16.0).astype(F8)
    decw[D, :NOUT] = (np.asarray(dec_b, dtype=np.float32) * 16.0).astype(F8)

    fcw = np.zeros((DP, E), dtype=BF)
    fcw[:D] = np.ascontiguousarray(
        np.asarray(attn_fc_w, dtype=np.float32).T).astype(BF)

    embt = np.asarray(emb_tim_w, dtype=np.float32).astype(BF)

    attnw = np.zeros((DP,), dtype=np.float32)
    attnw[:D] = np.asarray(attn_W, dtype=np.float32).reshape(-1)
    attnwr = np.repeat(attnw[:, None], 128, axis=1).astype(BF)

    uide = np.asarray(emb_uid_w, dtype=np.float32)[uid]
    fcb = np.asarray(attn_fc_b, dtype=np.float32).reshape(-1)
    constf = np.zeros((128, 9), dtype=np.float32)
    constf[:, 0] = np.arange(128, dtype=np.float32)
    constf[:, 1] = uide[0:128] * 64.0
    constf[:, 2] = uide[128:256] * 64.0
    constf[:, 3] = fcb[0:128]
    constf[:, 4] = fcb[128:256]
    constf[:, 5:9] = np.asarray(attn_b, dtype=np.float32).reshape(1, 4)

    in_maps = []
    for c in range(NCORES):
        r0 = c * R
        r1 = min(r0 + RH, S)
        n = r1 - r0
        appT = np.zeros((KAPPP, RP), dtype=F8)
        appT[:KAPP, :n] = app_f8[r0:r1].T

        timv = np.zeros((RP,), dtype=BF)
        timv[:n] = tim[r0:r1].astype(BF)

        ptimv = np.zeros((R,), dtype=BF)
        np_ = min(r0 + R, NWIN) - r0
        ptimv[:np_] = ptim[r0:r0 + np_].astype(BF)

        in_maps.append({
            "appT": appT, "wapp": wapp, "decw": decw, "fcw": fcw,
            "embt": embt, "attnwr": attnwr, "timv": timv,
            "ptimv": ptimv, "constf": constf,
        })
    return in_maps


def kernel(tim, app, loc, uid, ptim, emb_tim_w, emb_uid_w, emb_app_w,
           attn_W, attn_b, attn_fc_w, attn_fc_b, dec_w, dec_b,
           _trace=False, _trace_kwargs=None):
    if "nc" not in _CACHE:
        _CACHE["nc"] = _build()
    nc = _CACHE["nc"]

    in_maps = _host_prep(tim, app, uid, ptim, emb_tim_w, emb_uid_w, emb_app_w,
                         attn_W, attn_b, attn_fc_w, attn_fc_b, dec_w, dec_b)

    kw = {}
    if _trace:
        kw["trace"] = True
        if _trace_kwargs:
            kw.update(_trace_kwargs)
    res = bass_utils.run_bass_kernel_spmd(nc, in_maps, core_ids=list(range(NCORES)), **kw)
    _CACHE["last_result"] = res

    outs = []
    for c in range(NCORES):
        nrows = R if c < NCORES - 1 else NWIN - (NCORES - 1) * R
        outs.append(np.asarray(res.results[c]["out"])[:nrows, :NOUT])
    return np.concatenate(outs, axis=0).astype(np.float32)

